# revision 12
# baseline (speedup 1.0000x reference)
"""Pairwise cosine-distance matrix kernel for Trainium2 (Bass/Tile, 8 cores).

Problem: mapping [8192, 512] fp32 -> out[i,j] = 1 - <x_i, x_j> / (|x_i||x_j|),
full [8192, 8192] fp32 output.

Strategy (SPMD over 8 NeuronCores, symmetric-triangle partitioning):
  - The output is symmetric, so only the 136 distinct [512, 512] blocks of
    the 16x16 block grid need device compute. Circulant assignment: row-block
    r computes blocks (r, r+d mod 16) for d = 0..7, and the 8 "bridge"
    blocks (c, c+8) go one per core. Core c owns row-blocks c and c+8 ->
    exactly 17 blocks per core, with a uniform structure (weight A x 9
    column tiles + weight B x 8 column tiles) so a single NEFF serves all
    cores SPMD.
  - The host rotates the transposed matrix's columns by 512*c per core so
    that each core's weight/moving slices sit at identical compile-time
    offsets. Host input is fp16 (halves input DMA; the 2e-2 rel-err budget
    dwarfs fp16 quantization).
  - On device: per 512-column tile, compute column norms (ACT square ->
    DVE elementwise-sum of the 4 k-chunks -> ones-matmul partition reduce
    -> ACT rsqrt -> K=1 broadcast matmul) and scale the tile in place
    (fp16). The gram blocks run as fp16 matmuls (1 PE cycle/row) with fp32
    PSUM accumulation and a fused (1 - x) epilogue split across ACT/DVE
    into fp16 staging tiles, then DMA out.
  - Host upcasts to fp32, places each block, and mirrors its transpose to
    the symmetric position.
"""

import json
import os
import sys
import types

import numpy as np

N = 8192
D = 512
N_CORES = 8
NB = 16                 # 512-wide row/col blocks
BS = N // NB            # 512
KC = D // 128           # 4 k-chunks of 128
MT = BS // 128          # 4 row-chunks of 128 per 512-row part

# tiles normalized before gram group g runs (norm group g gates gram group g)
NORM_GROUPS = [[0, 1, 2], [3, 4, 5], [6, 7, 8], [9, 10], [11, 12, 13], [14, 15]]
# (weight_tile, moving_tiles, out_name, out_col_offset)
GRAM_GROUPS = [
    (0, [0, 1, 2], "outA", 0),
    (0, [3, 4, 5], "outA", 3 * BS),
    (0, [6, 7, 8], "outA", 6 * BS),
    (8, [8, 9, 10], "outB", 0),
    (8, [11, 12, 13], "outB", 3 * BS),
    (8, [14, 15], "outB", 6 * BS),
]

LAST_EXEC_NS = None  # max-across-traced-cores HW time of the last profiled run

_cached = {}


def _install_ntff_hook():
    """bass_utils' trace path imports antenv.axon_hooks, which this image
    lacks; recreate it and register the ctypes NTFF hook (same thing the
    boot script would have done)."""
    if "antenv.axon_hooks" in sys.modules:
        return
    mod = types.ModuleType("antenv.axon_hooks")
    holder = [None]
    mod.set_axon_ntff_profile_hook = lambda h: holder.__setitem__(0, h)
    mod.get_axon_ntff_profile_hook = lambda: holder[0]
    sys.modules["antenv.axon_hooks"] = mod
    import antenv
    antenv.axon_hooks = mod
    try:
        from trn_agent_boot.trn_boot import _ntff_profile_via_ctypes
        mod.set_axon_ntff_profile_hook(
            _ntff_profile_via_ctypes("/opt/axon/libaxon_pjrt.so")
        )
    except Exception:
        pass


def _split_multiwait_bir(bir_json: bytes) -> bytes:
    """This container's walrus rejects instructions with >1 semaphore wait
    ("Too many sync wait commands"). Hoist extra waits onto standalone
    wait-only EventSemaphore instructions placed just before, on the same
    engine — identical stall semantics."""
    m = json.loads(bir_json)
    for f in m["functions"]:
        for bb in f.get("blocks", f.get("basicblocks", [])):
            new_insts = []
            for inst in bb["instructions"]:
                si = inst.get("sync_info")
                waits = si.get("on_wait") if si else None
                if waits and len(waits) > 1:
                    for j, w in enumerate(waits[:-1]):
                        new_insts.append({
                            "debug": inst.get("debug"),
                            "engine": inst["engine"],
                            "ins": [],
                            "name": f"{inst['name']}-hw{j}",
                            "opcode": "EventSemaphore",
                            "outs": [],
                            "sync_info": {"on_update": [], "on_wait": [w]},
                        })
                    si["on_wait"] = [waits[-1]]
                new_insts.append(inst)
            bb["instructions"] = new_insts
    return json.dumps(m).encode()


def _apply_patches():
    if _cached.get("patched"):
        return
    _cached["patched"] = True
    import concourse.bass2jax as bass2jax
    import concourse.bass_utils as bass_utils

    orig_compile = bass2jax.compile_bir_kernel

    def patched_compile(bir_json, tmpdir, neff_name="file.neff"):
        return orig_compile(_split_multiwait_bir(bir_json), tmpdir,
                            neff_name=neff_name)

    bass2jax.compile_bir_kernel = patched_compile
    # No S3 in this container; the trace path uploads artifacts for links only.
    bass_utils.upload_artifacts = lambda tmpdir: "local://" + tmpdir


def _build():
    key = "nc"
    if key in _cached:
        return _cached[key]
    _apply_patches()
    import concourse.bass as bass
    import concourse.tile as tile
    from concourse import mybir

    f32 = mybir.dt.float32
    f16 = mybir.dt.float16
    Act = mybir.ActivationFunctionType
    Alu = mybir.AluOpType

    nc = bass.Bass(trn_type="TRN2", target_bir_lowering=False, debug=False)
    xt_d = nc.dram_tensor("xt", [D, N], f16, kind="ExternalInput").ap()
    outA_d = nc.dram_tensor("outA", [BS, 9 * BS], f16, kind="ExternalOutput").ap()
    outB_d = nc.dram_tensor("outB", [BS, 8 * BS], f16, kind="ExternalOutput").ap()
    # DRAM bounce buffers for the [1,512] <-> [128,4] norm reshuffles
    # (SBUF APs cannot remap free dim <-> partition dim; DRAM APs are flat)
    scr32_d = nc.dram_tensor("scr32", [N], f32, kind="Internal").ap()
    scr16_d = nc.dram_tensor("scr16", [N], f16, kind="Internal").ap()

    with tile.TileContext(nc) as tc:
        with (
            tc.tile_pool(name="xt", bufs=1) as xt_pool,
            tc.tile_pool(name="sq", bufs=6) as sq_pool,
            tc.tile_pool(name="tmp", bufs=10) as tmp_pool,
            tc.tile_pool(name="nrm", bufs=6) as nrm_pool,
            tc.tile_pool(name="rows", bufs=1) as row_pool,
            tc.tile_pool(name="bc16", bufs=5) as bc_pool,
            tc.tile_pool(name="consts", bufs=1) as const_pool,
            tc.tile_pool(name="stage", bufs=3) as stage_pool,
            tc.tile_pool(name="ps_nb", bufs=2, space=bass.MemorySpace.PSUM) as ps_nb,
            tc.tile_pool(name="ps_g", bufs=6, space=bass.MemorySpace.PSUM) as ps_g,
        ):
            ones_col = const_pool.tile([128, 1], f16, name="ones_col")
            nc.vector.memset(ones_col[:], 1.0)
            ones_row = const_pool.tile([1, 128], f16, name="ones_row")
            nc.vector.memset(ones_row[:], 1.0)
            one_bias = const_pool.tile([128, 1], f32, name="one_bias")
            nc.vector.memset(one_bias[:], 1.0)
            # rn_row[0, 128*w + p] = 1/||x_(128w+p)|| for the whole core
            rn_row = row_pool.tile([1, N], f16, name="rn_row")

            xt = [xt_pool.tile([128, N], f16, tag=f"xt{k}", name=f"xt{k}")
                  for k in range(KC)]
            # group-major input DMA so gram group 0's tiles land first;
            # group 0 is split per tile for faster first arrival
            for t in NORM_GROUPS[0]:
                for k in range(KC):
                    nc.sync.dma_start(
                        out=xt[k][:, t * BS:(t + 1) * BS],
                        in_=xt_d[k * 128:(k + 1) * 128, t * BS:(t + 1) * BS])
            for tiles in NORM_GROUPS[1:]:
                lo, hi = tiles[0] * BS, (tiles[-1] + 1) * BS
                for k in range(KC):
                    nc.sync.dma_start(out=xt[k][:, lo:hi],
                                      in_=xt_d[k * 128:(k + 1) * 128, lo:hi])

            def normalize(tiles, gid, lnexp=False):
                """Column-normalize the 512-wide slices `tiles` of xt in
                place (fp16): ACT square (f16) -> DVE 2x-mode adds ->
                ones-matmul partition reduce -> rsqrt -> K=1 broadcast
                matmul -> f16 copy -> 2x-mode scale.

                rsqrt path: bounce the [1,512] norm^2 rows through DRAM
                into a [128, 4/tile] layout so one full-width DVE
                reciprocal + ACT sqrt covers the whole group (single-
                partition DVE reciprocal costs ~6.3 ns/elem). For the
                startup group `lnexp` computes exp(-0.5*ln(x)) on ACT
                instead — ~6 us less chain latency, slightly more ACT."""
                W = 4 * len(tiles)
                c0 = tiles[0] * BS
                n2g = nrm_pool.tile([128, W], f32, tag="n2g", name=f"n2g_{gid}")
                for i, t in enumerate(tiles):
                    sl = slice(t * BS, (t + 1) * BS)
                    sqs = []
                    for k in range(KC):
                        sq = sq_pool.tile([128, BS], f16, tag="sq",
                                          name=f"sq{t}_{k}")
                        nc.scalar.square(sq[:], xt[k][:, sl])
                        sqs.append(sq)
                    a01 = tmp_pool.tile([128, BS], f16, tag="tmp",
                                        name=f"a01_{t}")
                    nc.vector.tensor_add(a01[:], sqs[0][:], sqs[1][:])
                    a23 = tmp_pool.tile([128, BS], f16, tag="tmp",
                                        name=f"a23_{t}")
                    nc.vector.tensor_add(a23[:], sqs[2][:], sqs[3][:])
                    ssum = tmp_pool.tile([128, BS], f16, tag="tmp",
                                         name=f"ssum_{t}")
                    nc.vector.tensor_add(ssum[:], a01[:], a23[:])
                    n2 = ps_nb.tile([1, BS], f32, tag="nb", name=f"n2_{t}")
                    nc.tensor.matmul(n2[:], ones_col[:], ssum[:],
                                     start=True, stop=True)
                    if lnexp:
                        lnx = tmp_pool.tile([1, BS], f32, tag="tmp",
                                            name=f"ln_{t}")
                        nc.scalar.activation(lnx[:], n2[:], Act.Ln)
                        nc.scalar.activation(rn_row[0:1, sl], lnx[:],
                                             Act.Exp, scale=-0.5)
                    else:
                        n2r = tmp_pool.tile([1, BS], f32, tag="tmp",
                                            name=f"n2r_{t}")
                        nc.scalar.copy(n2r[:], n2[:])  # DMA cannot read PSUM
                        nc.sync.dma_start(out=scr32_d[t * BS:(t + 1) * BS],
                                          in_=n2r[:])
                if not lnexp:
                    # scr32[128w+p+c0] -> n2g[p, w']: flat DRAM AP remaps
                    nc.sync.dma_start(
                        out=n2g[:],
                        in_=scr32_d[c0:c0 + W * 128].rearrange(
                            "(w p) -> p w", p=128))
                    rg = nrm_pool.tile([128, W], f32, tag="rg",
                                       name=f"rg_{gid}")
                    nc.vector.reciprocal(rg[:], n2g[:])
                    rs = nrm_pool.tile([128, W], f16, tag="rs",
                                       name=f"rs_{gid}")
                    nc.scalar.sqrt(rs[:], rg[:])
                    nc.sync.dma_start(
                        out=scr16_d[c0:c0 + W * 128].rearrange(
                            "(w p) -> p w", p=128),
                        in_=rs[:])
                    nc.sync.dma_start(
                        out=rn_row[0:1, c0:c0 + W * 128],
                        in_=scr16_d[c0:c0 + W * 128])
                for t in tiles:
                    sl = slice(t * BS, (t + 1) * BS)
                    bc = ps_nb.tile([128, BS], f32, tag="nb", name=f"bc_{t}")
                    nc.tensor.matmul(bc[:], ones_row[:], rn_row[0:1, sl],
                                     start=True, stop=True)
                    bc16 = bc_pool.tile([128, BS], f16, tag="bc",
                                        name=f"bc16_{t}")
                    nc.scalar.copy(bc16[:], bc[:])
                    for k in range(KC):
                        nc.vector.tensor_mul(xt[k][:, sl], xt[k][:, sl],
                                             bc16[:])

            normalize(NORM_GROUPS[0], 0, lnexp=True)
            normalize(NORM_GROUPS[1], 1)

            for gi, (wt, tiles, out_name, off) in enumerate(GRAM_GROUPS):
                out_d = outA_d if out_name == "outA" else outB_d
                nt = len(tiles)
                for mt in range(MT):
                    # prefetch norm group gi+2 under this group's gram
                    # stream (the DRAM-bounce chain is ~12-16 us; two
                    # gram groups of cover); high_priority floats it as
                    # early as deps allow
                    if mt == 0 and gi + 2 < len(NORM_GROUPS):
                        with tc.high_priority():
                            normalize(NORM_GROUPS[gi + 2], gi + 2)
                    psums = [ps_g.tile([128, BS], f32, tag="pg",
                                       name=f"pg_{gi}_{mt}_{j}")
                             for j in range(nt)]
                    for k in range(KC):
                        w = xt[k][:, wt * BS + mt * 128:wt * BS + mt * 128 + 128]
                        for j, t in enumerate(tiles):
                            nc.tensor.matmul(psums[j][:], w,
                                             xt[k][:, t * BS:(t + 1) * BS],
                                             start=(k == 0), stop=(k == KC - 1))
                    stage = stage_pool.tile([128, nt * BS], f16, tag="st",
                                            name=f"st_{gi}_{mt}")
                    for j in range(nt):
                        ssl = slice(j * BS, (j + 1) * BS)
                        # ~1/3 of epilogue converts on DVE, rest on ACT
                        if (gi * MT + mt + j) % 3 == 0:
                            nc.vector.tensor_scalar(stage[:, ssl], psums[j][:],
                                                    -1.0, 1.0,
                                                    Alu.mult, Alu.add)
                        else:
                            nc.scalar.activation(stage[:, ssl], psums[j][:],
                                                 Act.Identity,
                                                 bias=one_bias[:], scale=-1.0)
                    nc.sync.dma_start(
                        out=out_d[mt * 128:(mt + 1) * 128, off:off + nt * BS],
                        in_=stage[:])

    _cached[key] = nc
    return nc


def kernel(mapping: np.ndarray) -> np.ndarray:
    from concourse.bass_utils import run_bass_kernel_spmd

    mapping = np.ascontiguousarray(mapping, dtype=np.float32)
    assert mapping.shape == (N, D)
    xt16 = np.ascontiguousarray(mapping.T.astype(np.float16))  # [512, 8192]
    in_maps = []
    for c in range(N_CORES):
        in_maps.append({"xt": np.ascontiguousarray(
            np.roll(xt16, -BS * c, axis=1))})

    nc = _build()

    trace = bool(int(os.environ.get("BASSKNN_TRACE", "0")))
    if trace:
        _install_ntff_hook()
    res = run_bass_kernel_spmd(nc, in_maps, list(range(N_CORES)), trace=trace)
    global LAST_EXEC_NS
    if trace:
        LAST_EXEC_NS = res.exec_time_ns

    full = np.empty((N, N), np.float32)
    for c in range(N_CORES):
        A = np.asarray(res.results[c]["outA"]).astype(np.float32)
        B = np.asarray(res.results[c]["outB"]).astype(np.float32)
        for t in range(9):
            j = (c + t) % NB
            blk = A[:, t * BS:(t + 1) * BS]
            full[c * BS:(c + 1) * BS, j * BS:(j + 1) * BS] = blk
            if t:
                full[j * BS:(j + 1) * BS, c * BS:(c + 1) * BS] = blk.T
        i2 = c + 8
        for e in range(8):
            j = (i2 + e) % NB
            blk = B[:, e * BS:(e + 1) * BS]
            full[i2 * BS:(i2 + 1) * BS, j * BS:(j + 1) * BS] = blk
            if e:
                full[j * BS:(j + 1) * BS, i2 * BS:(i2 + 1) * BS] = blk.T
    return full


# revision 17
# speedup vs baseline: 1.0082x; 1.0082x over previous
"""Pairwise cosine-distance matrix kernel for Trainium2 (Bass/Tile, 8 cores).

Problem: mapping [8192, 512] fp32 -> out[i,j] = 1 - <x_i, x_j> / (|x_i||x_j|),
full [8192, 8192] fp32 output.

Strategy (SPMD over 8 NeuronCores, symmetric-triangle partitioning):
  - The output is symmetric, so only the 136 distinct [512, 512] blocks of
    the 16x16 block grid need device compute. Circulant assignment: row-block
    r computes blocks (r, r+d mod 16) for d = 0..7, and the 8 "bridge"
    blocks (c, c+8) go one per core. Core c owns row-blocks c and c+8 ->
    exactly 17 blocks per core, with a uniform structure (weight A x 9
    column tiles + weight B x 8 column tiles) so a single NEFF serves all
    cores SPMD.
  - The host rotates the transposed matrix's columns by 512*c per core so
    that each core's weight/moving slices sit at identical compile-time
    offsets. Host input is fp16 (halves input DMA; the 2e-2 rel-err budget
    dwarfs fp16 quantization).
  - On device: per 512-column tile, compute column norms (ACT square ->
    DVE elementwise-sum of the 4 k-chunks -> ones-matmul partition reduce
    -> ACT rsqrt -> K=1 broadcast matmul) and scale the tile in place
    (fp16). The gram blocks run as fp16 matmuls (1 PE cycle/row) with fp32
    PSUM accumulation and a fused (1 - x) epilogue split across ACT/DVE
    into fp16 staging tiles, then DMA out.
  - Host upcasts to fp32, places each block, and mirrors its transpose to
    the symmetric position.
"""

import json
import os
import sys
import types

import numpy as np

N = 8192
D = 512
N_CORES = 8
NB = 16                 # 512-wide row/col blocks
BS = N // NB            # 512
KC = D // 128           # 4 k-chunks of 128
MT = BS // 128          # 4 row-chunks of 128 per 512-row part

# tiles normalized before gram group g runs (norm group g gates gram group g)
NORM_GROUPS = [[0, 1, 2], [3, 4, 5], [6, 7, 8], [9, 10], [11, 12, 13], [14, 15]]
# (weight_tile, moving_tiles, out_name, out_col_offset)
GRAM_GROUPS = [
    (0, [0, 1, 2], "outA", 0),
    (0, [3, 4, 5], "outA", 3 * BS),
    (0, [6, 7, 8], "outA", 6 * BS),
    (8, [8, 9, 10], "outB", 0),
    (8, [11, 12, 13], "outB", 3 * BS),
    (8, [14, 15], "outB", 6 * BS),
]

LAST_EXEC_NS = None  # max-across-traced-cores HW time of the last profiled run

_cached = {}


def _install_ntff_hook():
    """bass_utils' trace path imports antenv.axon_hooks, which this image
    lacks; recreate it and register the ctypes NTFF hook (same thing the
    boot script would have done)."""
    if "antenv.axon_hooks" in sys.modules:
        return
    mod = types.ModuleType("antenv.axon_hooks")
    holder = [None]
    mod.set_axon_ntff_profile_hook = lambda h: holder.__setitem__(0, h)
    mod.get_axon_ntff_profile_hook = lambda: holder[0]
    sys.modules["antenv.axon_hooks"] = mod
    import antenv
    antenv.axon_hooks = mod
    try:
        from trn_agent_boot.trn_boot import _ntff_profile_via_ctypes
        mod.set_axon_ntff_profile_hook(
            _ntff_profile_via_ctypes("/opt/axon/libaxon_pjrt.so")
        )
    except Exception:
        pass


def _split_multiwait_bir(bir_json: bytes) -> bytes:
    """This container's walrus rejects instructions with >1 semaphore wait
    ("Too many sync wait commands"). Hoist extra waits onto standalone
    wait-only EventSemaphore instructions placed just before, on the same
    engine — identical stall semantics."""
    m = json.loads(bir_json)
    for f in m["functions"]:
        for bb in f.get("blocks", f.get("basicblocks", [])):
            new_insts = []
            for inst in bb["instructions"]:
                si = inst.get("sync_info")
                waits = si.get("on_wait") if si else None
                if waits and len(waits) > 1:
                    for j, w in enumerate(waits[:-1]):
                        new_insts.append({
                            "debug": inst.get("debug"),
                            "engine": inst["engine"],
                            "ins": [],
                            "name": f"{inst['name']}-hw{j}",
                            "opcode": "EventSemaphore",
                            "outs": [],
                            "sync_info": {"on_update": [], "on_wait": [w]},
                        })
                    si["on_wait"] = [waits[-1]]
                new_insts.append(inst)
            bb["instructions"] = new_insts
    return json.dumps(m).encode()


def _apply_patches():
    if _cached.get("patched"):
        return
    _cached["patched"] = True
    import concourse.bass2jax as bass2jax
    import concourse.bass_utils as bass_utils

    orig_compile = bass2jax.compile_bir_kernel

    def patched_compile(bir_json, tmpdir, neff_name="file.neff"):
        return orig_compile(_split_multiwait_bir(bir_json), tmpdir,
                            neff_name=neff_name)

    bass2jax.compile_bir_kernel = patched_compile
    # No S3 in this container; the trace path uploads artifacts for links only.
    bass_utils.upload_artifacts = lambda tmpdir: "local://" + tmpdir


def _build():
    key = "nc"
    if key in _cached:
        return _cached[key]
    _apply_patches()
    import concourse.bass as bass
    import concourse.tile as tile
    from concourse import mybir

    f32 = mybir.dt.float32
    f16 = mybir.dt.float16
    Act = mybir.ActivationFunctionType
    Alu = mybir.AluOpType

    nc = bass.Bass(trn_type="TRN2", target_bir_lowering=False, debug=False)
    xt_d = nc.dram_tensor("xt", [D, N], f16, kind="ExternalInput").ap()
    outA_d = nc.dram_tensor("outA", [BS, 9 * BS], f16, kind="ExternalOutput").ap()
    outB_d = nc.dram_tensor("outB", [BS, 8 * BS], f16, kind="ExternalOutput").ap()
    # Per-group DRAM bounce buffers for the [1,512] <-> [128,4] norm
    # reshuffles (SBUF APs cannot remap free dim <-> partition dim; DRAM
    # APs are flat). Separate tensors per group avoid cross-group deps.
    scr32_d = [nc.dram_tensor(f"scr32_{g}", [4 * BS], f32, kind="Internal").ap()
               for g in range(len(NORM_GROUPS))]
    scr16_d = [nc.dram_tensor(f"scr16_{g}", [4 * BS], f16, kind="Internal").ap()
               for g in range(len(NORM_GROUPS))]

    with tile.TileContext(nc) as tc:
        with (
            tc.tile_pool(name="xt", bufs=1) as xt_pool,
            tc.tile_pool(name="sq", bufs=3) as sq_pool,
            tc.tile_pool(name="tmp", bufs=8) as tmp_pool,
            tc.tile_pool(name="nrm", bufs=6) as nrm_pool,
            tc.tile_pool(name="rows", bufs=3) as row_pool,
            tc.tile_pool(name="bc16", bufs=5) as bc_pool,
            tc.tile_pool(name="consts", bufs=1) as const_pool,
            tc.tile_pool(name="stage", bufs=3) as stage_pool,
            tc.tile_pool(name="ps_nb", bufs=2, space=bass.MemorySpace.PSUM) as ps_nb,
            tc.tile_pool(name="ps_g", bufs=6, space=bass.MemorySpace.PSUM) as ps_g,
        ):
            ones_col = const_pool.tile([128, 1], f16, name="ones_col")
            nc.vector.memset(ones_col[:], 1.0)
            ones_row = const_pool.tile([1, 128], f16, name="ones_row")
            nc.vector.memset(ones_row[:], 1.0)
            one_bias = const_pool.tile([128, 1], f32, name="one_bias")
            nc.vector.memset(one_bias[:], 1.0)

            # single fused tile, k-chunk-major: chunk k col j at k*N + j.
            # Lets the per-512-col-tile square/scale run as ONE wide DVE op
            # over a [128, 4, 512] strided view instead of 4 ops.
            xt = xt_pool.tile([128, KC * N], f16, name="xt")
            xt_v = xt[:].rearrange("p (a c) -> p a c", a=KC)

            def xk(k):
                return xt[:, k * N:(k + 1) * N]

            # group-major input DMA so gram group 0's tiles land first;
            # group 0 is split per tile for faster first arrival
            for t in NORM_GROUPS[0]:
                for k in range(KC):
                    nc.sync.dma_start(
                        out=xk(k)[:, t * BS:(t + 1) * BS],
                        in_=xt_d[k * 128:(k + 1) * 128, t * BS:(t + 1) * BS])
            for tiles in NORM_GROUPS[1:]:
                lo, hi = tiles[0] * BS, (tiles[-1] + 1) * BS
                for k in range(KC):
                    nc.sync.dma_start(out=xk(k)[:, lo:hi],
                                      in_=xt_d[k * 128:(k + 1) * 128, lo:hi])

            def normalize(tiles, gid, lnexp=False):
                """Column-normalize the 512-wide slices `tiles` of xt in
                place (fp16): fused DVE square over the 4 k-chunks ->
                2-level DVE adds -> ones-matmul partition reduce -> rsqrt
                -> K=1 broadcast matmul -> f16 copy -> fused DVE scale.

                rsqrt path: bounce the [1,512] norm^2 rows through DRAM
                into a [128, 4/tile] layout so one full-width DVE
                reciprocal + ACT sqrt covers the whole group (single-
                partition DVE reciprocal costs ~6.3 ns/elem). These
                bounce DMAs issue from the otherwise-idle GpSimd queue so
                their waits cannot convoy the Sync queue's input/output
                DMAs. For the startup group `lnexp` computes
                exp(-0.5*ln(x)) on ACT instead — shorter chain."""
                W = 4 * len(tiles)
                rn = row_pool.tile([1, len(tiles) * BS], f16, tag="rn",
                                   name=f"rn_{gid}")
                n2g = nrm_pool.tile([128, W], f32, tag="n2g", name=f"n2g_{gid}")
                for i, t in enumerate(tiles):
                    sl = slice(t * BS, (t + 1) * BS)
                    sq = sq_pool.tile([128, KC * BS], f16, tag="sq",
                                      name=f"sq{t}")
                    nc.vector.tensor_mul(
                        sq[:].rearrange("p (a c) -> p a c", a=KC),
                        xt_v[:, :, sl], xt_v[:, :, sl])
                    a01 = tmp_pool.tile([128, 2 * BS], f16, tag="tmp",
                                        name=f"a01_{t}")
                    nc.vector.tensor_add(a01[:], sq[:, 0:2 * BS],
                                         sq[:, 2 * BS:4 * BS])
                    ssum = tmp_pool.tile([128, BS], f16, tag="tmp",
                                         name=f"ssum_{t}")
                    nc.vector.tensor_add(ssum[:], a01[:, 0:BS], a01[:, BS:])
                    n2 = ps_nb.tile([1, BS], f32, tag="nb", name=f"n2_{t}")
                    nc.tensor.matmul(n2[:], ones_col[:], ssum[:],
                                     start=True, stop=True)
                    if lnexp:
                        lnx = tmp_pool.tile([1, BS], f32, tag="tmp",
                                            name=f"ln_{t}")
                        nc.scalar.activation(lnx[:], n2[:], Act.Ln)
                        nc.scalar.activation(rn[0:1, i * BS:(i + 1) * BS],
                                             lnx[:], Act.Exp, scale=-0.5)
                    else:
                        n2r = tmp_pool.tile([1, BS], f32, tag="tmp",
                                            name=f"n2r_{t}")
                        nc.scalar.copy(n2r[:], n2[:])  # DMA cannot read PSUM
                        nc.gpsimd.dma_start(
                            out=scr32_d[gid][i * BS:(i + 1) * BS], in_=n2r[:])
                if not lnexp:
                    # scr32[128w+p] -> n2g[p, w]: flat DRAM AP remaps
                    nc.gpsimd.dma_start(
                        out=n2g[:],
                        in_=scr32_d[gid][0:W * 128].rearrange(
                            "(w p) -> p w", p=128))
                    rg = nrm_pool.tile([128, W], f32, tag="rg",
                                       name=f"rg_{gid}")
                    nc.vector.reciprocal(rg[:], n2g[:])
                    rs = nrm_pool.tile([128, W], f16, tag="rs",
                                       name=f"rs_{gid}")
                    nc.scalar.sqrt(rs[:], rg[:])
                    nc.gpsimd.dma_start(
                        out=scr16_d[gid][0:W * 128].rearrange(
                            "(w p) -> p w", p=128),
                        in_=rs[:])
                    nc.gpsimd.dma_start(out=rn[:],
                                        in_=scr16_d[gid][0:W * 128])
                for i, t in enumerate(tiles):
                    sl = slice(t * BS, (t + 1) * BS)
                    bc = ps_nb.tile([128, BS], f32, tag="nb", name=f"bc_{t}")
                    nc.tensor.matmul(bc[:], ones_row[:],
                                     rn[0:1, i * BS:(i + 1) * BS],
                                     start=True, stop=True)
                    bc16 = bc_pool.tile([128, BS], f16, tag="bc",
                                        name=f"bc16_{t}")
                    nc.scalar.copy(bc16[:], bc[:])
                    nc.vector.tensor_mul(
                        xt_v[:, :, sl], xt_v[:, :, sl],
                        bc16[:].unsqueeze(1).broadcast_to((128, KC, BS)))

            normalize(NORM_GROUPS[0], 0, lnexp=True)
            normalize(NORM_GROUPS[1], 1)

            for gi, (wt, tiles, out_name, off) in enumerate(GRAM_GROUPS):
                out_d = outA_d if out_name == "outA" else outB_d
                nt = len(tiles)
                for mt in range(MT):
                    # prefetch norm group gi+2 under this group's gram
                    # stream (the DRAM-bounce chain is ~12-16 us; two
                    # gram groups of cover); high_priority floats it as
                    # early as deps allow
                    if mt == 0 and gi + 2 < len(NORM_GROUPS):
                        with tc.high_priority():
                            normalize(NORM_GROUPS[gi + 2], gi + 2)
                    psums = [ps_g.tile([128, BS], f32, tag="pg",
                                       name=f"pg_{gi}_{mt}_{j}")
                             for j in range(nt)]
                    for k in range(KC):
                        w = xk(k)[:, wt * BS + mt * 128:wt * BS + mt * 128 + 128]
                        for j, t in enumerate(tiles):
                            nc.tensor.matmul(psums[j][:], w,
                                             xk(k)[:, t * BS:(t + 1) * BS],
                                             start=(k == 0), stop=(k == KC - 1))
                    stage = stage_pool.tile([128, nt * BS], f16, tag="st",
                                            name=f"st_{gi}_{mt}")
                    for j in range(nt):
                        ssl = slice(j * BS, (j + 1) * BS)
                        # DVE is loaded with the norm chain; put ~1/10 of
                        # the epilogue converts there, the rest on ACT
                        if (gi * MT + mt + j) % 10 == 0:
                            nc.vector.tensor_scalar(stage[:, ssl], psums[j][:],
                                                    -1.0, 1.0,
                                                    Alu.mult, Alu.add)
                        else:
                            nc.scalar.activation(stage[:, ssl], psums[j][:],
                                                 Act.Identity,
                                                 bias=one_bias[:], scale=-1.0)
                    nc.sync.dma_start(
                        out=out_d[mt * 128:(mt + 1) * 128, off:off + nt * BS],
                        in_=stage[:])

    _cached[key] = nc
    return nc


def kernel(mapping: np.ndarray) -> np.ndarray:
    from concourse.bass_utils import run_bass_kernel_spmd

    mapping = np.ascontiguousarray(mapping, dtype=np.float32)
    assert mapping.shape == (N, D)
    xt16 = np.ascontiguousarray(mapping.T.astype(np.float16))  # [512, 8192]
    in_maps = []
    for c in range(N_CORES):
        in_maps.append({"xt": np.ascontiguousarray(
            np.roll(xt16, -BS * c, axis=1))})

    nc = _build()

    trace = bool(int(os.environ.get("BASSKNN_TRACE", "0")))
    if trace:
        _install_ntff_hook()
    res = run_bass_kernel_spmd(nc, in_maps, list(range(N_CORES)), trace=trace)
    global LAST_EXEC_NS
    if trace:
        LAST_EXEC_NS = res.exec_time_ns

    full = np.empty((N, N), np.float32)
    for c in range(N_CORES):
        A = np.asarray(res.results[c]["outA"]).astype(np.float32)
        B = np.asarray(res.results[c]["outB"]).astype(np.float32)
        for t in range(9):
            j = (c + t) % NB
            blk = A[:, t * BS:(t + 1) * BS]
            full[c * BS:(c + 1) * BS, j * BS:(j + 1) * BS] = blk
            if t:
                full[j * BS:(j + 1) * BS, c * BS:(c + 1) * BS] = blk.T
        i2 = c + 8
        for e in range(8):
            j = (i2 + e) % NB
            blk = B[:, e * BS:(e + 1) * BS]
            full[i2 * BS:(i2 + 1) * BS, j * BS:(j + 1) * BS] = blk
            if e:
                full[j * BS:(j + 1) * BS, i2 * BS:(i2 + 1) * BS] = blk.T
    return full


# revision 19
# speedup vs baseline: 1.2576x; 1.2473x over previous
"""Pairwise cosine-distance matrix kernel for Trainium2 (Bass/Tile, 8 cores).

Problem: mapping [8192, 512] fp32 -> out[i,j] = 1 - <x_i, x_j> / (|x_i||x_j|),
full [8192, 8192] fp32 output.

Strategy (SPMD over 8 NeuronCores, symmetric-triangle partitioning):
  - The output is symmetric, so only the 136 distinct [512, 512] blocks of
    the 16x16 block grid need device compute. Circulant assignment: row-block
    r computes blocks (r, r+d mod 16) for d = 0..7, and the 8 "bridge"
    blocks (c, c+8) go one per core. Core c owns row-blocks c and c+8 ->
    exactly 17 blocks per core, with a uniform structure (weight A x 9
    column tiles + weight B x 8 column tiles) so a single NEFF serves all
    cores SPMD.
  - The host rotates the transposed matrix's columns by 512*c per core so
    that each core's weight/moving slices sit at identical compile-time
    offsets. Host input is fp16 (halves input DMA; the 2e-2 rel-err budget
    dwarfs fp16 quantization).
  - On device: per 512-column tile, compute column norms (ACT square ->
    DVE elementwise-sum of the 4 k-chunks -> ones-matmul partition reduce
    -> ACT rsqrt -> K=1 broadcast matmul) and scale the tile in place
    (fp16). The gram blocks run as fp16 matmuls (1 PE cycle/row) with fp32
    PSUM accumulation and a fused (1 - x) epilogue split across ACT/DVE
    into fp16 staging tiles, then DMA out.
  - Host upcasts to fp32, places each block, and mirrors its transpose to
    the symmetric position.
"""

import json
import os
import sys
import types

import numpy as np

N = 8192
D = 512
N_CORES = 8
NB = 16                 # 512-wide row/col blocks
BS = N // NB            # 512
KC = D // 128           # 4 k-chunks of 128
MT = BS // 128          # 4 row-chunks of 128 per 512-row part

# tiles normalized before gram group g runs (norm group g gates gram group g)
NORM_GROUPS = [[0, 1, 2], [3, 4, 5], [6, 7, 8], [9, 10], [11, 12, 13], [14, 15]]
# (weight_tile, moving_tiles, out_name, out_col_offset)
GRAM_GROUPS = [
    (0, [0, 1, 2], "outA", 0),
    (0, [3, 4, 5], "outA", 3 * BS),
    (0, [6, 7, 8], "outA", 6 * BS),
    (8, [8, 9, 10], "outB", 0),
    (8, [11, 12, 13], "outB", 3 * BS),
    (8, [14, 15], "outB", 6 * BS),
]

LAST_EXEC_NS = None  # max-across-traced-cores HW time of the last profiled run

_cached = {}


def _install_ntff_hook():
    """bass_utils' trace path imports antenv.axon_hooks, which this image
    lacks; recreate it and register the ctypes NTFF hook (same thing the
    boot script would have done)."""
    if "antenv.axon_hooks" in sys.modules:
        return
    mod = types.ModuleType("antenv.axon_hooks")
    holder = [None]
    mod.set_axon_ntff_profile_hook = lambda h: holder.__setitem__(0, h)
    mod.get_axon_ntff_profile_hook = lambda: holder[0]
    sys.modules["antenv.axon_hooks"] = mod
    import antenv
    antenv.axon_hooks = mod
    try:
        from trn_agent_boot.trn_boot import _ntff_profile_via_ctypes
        mod.set_axon_ntff_profile_hook(
            _ntff_profile_via_ctypes("/opt/axon/libaxon_pjrt.so")
        )
    except Exception:
        pass


def _split_multiwait_bir(bir_json: bytes) -> bytes:
    """This container's walrus rejects instructions with >1 semaphore wait
    ("Too many sync wait commands"). Hoist extra waits onto standalone
    wait-only EventSemaphore instructions placed just before, on the same
    engine — identical stall semantics."""
    m = json.loads(bir_json)
    for f in m["functions"]:
        for bb in f.get("blocks", f.get("basicblocks", [])):
            new_insts = []
            for inst in bb["instructions"]:
                si = inst.get("sync_info")
                waits = si.get("on_wait") if si else None
                if waits and len(waits) > 1:
                    for j, w in enumerate(waits[:-1]):
                        new_insts.append({
                            "debug": inst.get("debug"),
                            "engine": inst["engine"],
                            "ins": [],
                            "name": f"{inst['name']}-hw{j}",
                            "opcode": "EventSemaphore",
                            "outs": [],
                            "sync_info": {"on_update": [], "on_wait": [w]},
                        })
                    si["on_wait"] = [waits[-1]]
                new_insts.append(inst)
            bb["instructions"] = new_insts
    return json.dumps(m).encode()


def _apply_patches():
    if _cached.get("patched"):
        return
    _cached["patched"] = True
    import concourse.bass2jax as bass2jax
    import concourse.bass_utils as bass_utils

    orig_compile = bass2jax.compile_bir_kernel

    def patched_compile(bir_json, tmpdir, neff_name="file.neff"):
        return orig_compile(_split_multiwait_bir(bir_json), tmpdir,
                            neff_name=neff_name)

    bass2jax.compile_bir_kernel = patched_compile
    # No S3 in this container; the trace path uploads artifacts for links only.
    bass_utils.upload_artifacts = lambda tmpdir: "local://" + tmpdir


def _build():
    key = "nc"
    if key in _cached:
        return _cached[key]
    _apply_patches()
    import concourse.bass as bass
    import concourse.tile as tile
    from concourse import mybir

    f32 = mybir.dt.float32
    f16 = mybir.dt.float16
    Act = mybir.ActivationFunctionType
    Alu = mybir.AluOpType

    nc = bass.Bass(trn_type="TRN2", target_bir_lowering=False, debug=False)
    xt_d = nc.dram_tensor("xt", [D, N], f16, kind="ExternalInput").ap()
    outA_d = nc.dram_tensor("outA", [BS, 9 * BS], f16, kind="ExternalOutput").ap()
    outB_d = nc.dram_tensor("outB", [BS, 8 * BS], f16, kind="ExternalOutput").ap()
    # Per-group DRAM bounce buffers for the [1,512] <-> [128,4] norm
    # reshuffles (SBUF APs cannot remap free dim <-> partition dim; DRAM
    # APs are flat). Separate tensors per group avoid cross-group deps.
    scr32_d = [nc.dram_tensor(f"scr32_{g}", [4 * BS], f32, kind="Internal").ap()
               for g in range(len(NORM_GROUPS))]
    scr16_d = [nc.dram_tensor(f"scr16_{g}", [4 * BS], f16, kind="Internal").ap()
               for g in range(len(NORM_GROUPS))]

    with tile.TileContext(nc) as tc:
        with (
            tc.tile_pool(name="xt", bufs=1) as xt_pool,
            tc.tile_pool(name="sq", bufs=3) as sq_pool,
            tc.tile_pool(name="tmp", bufs=8) as tmp_pool,
            tc.tile_pool(name="nrm", bufs=6) as nrm_pool,
            tc.tile_pool(name="rows", bufs=2) as row_pool,
            tc.tile_pool(name="bc16", bufs=5) as bc_pool,
            tc.tile_pool(name="consts", bufs=1) as const_pool,
            tc.tile_pool(name="stage", bufs=3) as stage_pool,
            tc.tile_pool(name="ps_nb", bufs=2, space=bass.MemorySpace.PSUM) as ps_nb,
            tc.tile_pool(name="ps_g", bufs=6, space=bass.MemorySpace.PSUM) as ps_g,
        ):
            ones_col = const_pool.tile([128, 1], f16, name="ones_col")
            nc.vector.memset(ones_col[:], 1.0)
            one_bias = const_pool.tile([128, 1], f32, name="one_bias")
            nc.vector.memset(one_bias[:], 1.0)

            # xt is tile-major: 512-col tile t occupies [2048t, 2048(t+1)),
            # with k-chunk k at +512k. Every op below then works on plain
            # contiguous 2-dim slices (exact ranges for dep tracking), and
            # the per-tile square/scale run as ONE wide fp16 2x-mode DVE op.
            TW = KC * BS                      # 2048 cols per tile
            xt = xt_pool.tile([128, NB * TW], f16, name="xt")

            def xtile(t):
                return xt[:, TW * t:TW * (t + 1)]

            def xmov(k, t):
                return xt[:, TW * t + BS * k:TW * t + BS * (k + 1)]

            def xw(k, wt, mt):
                base = TW * wt + BS * k + 128 * mt
                return xt[:, base:base + 128]

            # tile-major input DMA, norm-group order
            for tiles in NORM_GROUPS:
                for t in tiles:
                    for k in range(KC):
                        nc.sync.dma_start(
                            out=xmov(k, t),
                            in_=xt_d[k * 128:(k + 1) * 128,
                                     t * BS:(t + 1) * BS])

            def normalize(tiles, gid, lnexp=False):
                """Column-normalize the 512-wide tiles in place (fp16):
                one fused DVE square per tile -> 2-level DVE adds ->
                ones-matmul partition reduce -> rsqrt -> broadcast-DMA the
                1/norm row from DRAM into a [128,512] tile -> one fused
                DVE scale.

                rsqrt path: bounce the [1,512] norm^2 rows through DRAM
                into a [128, 4/tile] layout so one full-width DVE
                reciprocal + ACT sqrt covers the whole group (single-
                partition DVE reciprocal costs ~6.3 ns/elem). All bounce
                DMAs ride the otherwise-idle GpSimd queue so their waits
                cannot convoy the Sync queue; the broadcast comes straight
                from DRAM so NO PE matmul sits in the norm chain (a
                stalled PE instruction blocks the whole FIFO queue). For
                the startup group `lnexp` computes exp(-0.5*ln(x)) on ACT
                instead - a shorter chain."""
                W = 4 * len(tiles)
                n2g = nrm_pool.tile([128, W], f32, tag="n2g", name=f"n2g_{gid}")
                rn = row_pool.tile([1, len(tiles) * BS], f16, tag="rn",
                                   name=f"rn_{gid}")
                for i, t in enumerate(tiles):
                    sq = sq_pool.tile([128, TW], f16, tag="sq", name=f"sq{t}")
                    nc.vector.tensor_mul(sq[:], xtile(t), xtile(t))
                    a01 = tmp_pool.tile([128, 2 * BS], f16, tag="tmp",
                                        name=f"a01_{t}")
                    nc.vector.tensor_add(a01[:], sq[:, 0:2 * BS],
                                         sq[:, 2 * BS:4 * BS])
                    ssum = tmp_pool.tile([128, BS], f16, tag="tmp",
                                         name=f"ssum_{t}")
                    nc.vector.tensor_add(ssum[:], a01[:, 0:BS], a01[:, BS:])
                    n2 = ps_nb.tile([1, BS], f32, tag="nb", name=f"n2_{t}")
                    nc.tensor.matmul(n2[:], ones_col[:], ssum[:],
                                     start=True, stop=True)
                    if lnexp:
                        lnx = tmp_pool.tile([1, BS], f32, tag="tmp",
                                            name=f"ln_{t}")
                        nc.scalar.activation(lnx[:], n2[:], Act.Ln)
                        nc.scalar.activation(rn[0:1, i * BS:(i + 1) * BS],
                                             lnx[:], Act.Exp, scale=-0.5)
                        nc.gpsimd.dma_start(
                            out=scr16_d[gid][i * BS:(i + 1) * BS],
                            in_=rn[0:1, i * BS:(i + 1) * BS])
                    else:
                        n2r = tmp_pool.tile([1, BS], f32, tag="tmp",
                                            name=f"n2r_{t}")
                        nc.scalar.copy(n2r[:], n2[:])  # DMA cannot read PSUM
                        nc.gpsimd.dma_start(
                            out=scr32_d[gid][i * BS:(i + 1) * BS], in_=n2r[:])
                if not lnexp:
                    # scr32[128w+p] -> n2g[p, w]: flat DRAM AP remaps
                    nc.gpsimd.dma_start(
                        out=n2g[:],
                        in_=scr32_d[gid][0:W * 128].rearrange(
                            "(w p) -> p w", p=128))
                    rg = nrm_pool.tile([128, W], f32, tag="rg",
                                       name=f"rg_{gid}")
                    nc.vector.reciprocal(rg[:], n2g[:])
                    rs = nrm_pool.tile([128, W], f16, tag="rs",
                                       name=f"rs_{gid}")
                    nc.scalar.sqrt(rs[:], rg[:])
                    nc.gpsimd.dma_start(
                        out=scr16_d[gid][0:W * 128].rearrange(
                            "(w p) -> p w", p=128),
                        in_=rs[:])
                for i, t in enumerate(tiles):
                    bc16 = bc_pool.tile([128, BS], f16, tag="bc",
                                        name=f"bc16_{t}")
                    nc.gpsimd.dma_start(
                        out=bc16[:],
                        in_=scr16_d[gid][i * BS:(i + 1) * BS]
                        .unsqueeze(0).broadcast_to((128, BS)))
                    nc.vector.tensor_mul(
                        xtile(t).rearrange("p (a c) -> p a c", a=KC),
                        xtile(t).rearrange("p (a c) -> p a c", a=KC),
                        bc16[:].unsqueeze(1).broadcast_to((128, KC, BS)))

            normalize(NORM_GROUPS[0], 0, lnexp=True)
            normalize(NORM_GROUPS[1], 1)

            for gi, (wt, tiles, out_name, off) in enumerate(GRAM_GROUPS):
                out_d = outA_d if out_name == "outA" else outB_d
                nt = len(tiles)
                for mt in range(MT):
                    # prefetch norm group gi+2 under this group's gram
                    # stream; high_priority floats it as early as deps allow
                    if mt == 0 and gi + 2 < len(NORM_GROUPS):
                        with tc.high_priority():
                            normalize(NORM_GROUPS[gi + 2], gi + 2)
                    psums = [ps_g.tile([128, BS], f32, tag="pg",
                                       name=f"pg_{gi}_{mt}_{j}")
                             for j in range(nt)]
                    for k in range(KC):
                        for j, t in enumerate(tiles):
                            nc.tensor.matmul(psums[j][:], xw(k, wt, mt),
                                             xmov(k, t),
                                             start=(k == 0), stop=(k == KC - 1))
                    stage = stage_pool.tile([128, nt * BS], f16, tag="st",
                                            name=f"st_{gi}_{mt}")
                    for j in range(nt):
                        # epilogue all on ACT: DVE carries the norm chain
                        nc.scalar.activation(stage[:, j * BS:(j + 1) * BS],
                                             psums[j][:], Act.Identity,
                                             bias=one_bias[:], scale=-1.0)
                    nc.sync.dma_start(
                        out=out_d[mt * 128:(mt + 1) * 128, off:off + nt * BS],
                        in_=stage[:])

    _cached[key] = nc
    return nc


def kernel(mapping: np.ndarray) -> np.ndarray:
    from concourse.bass_utils import run_bass_kernel_spmd

    mapping = np.ascontiguousarray(mapping, dtype=np.float32)
    assert mapping.shape == (N, D)
    xt16 = np.ascontiguousarray(mapping.T.astype(np.float16))  # [512, 8192]
    in_maps = []
    for c in range(N_CORES):
        in_maps.append({"xt": np.ascontiguousarray(
            np.roll(xt16, -BS * c, axis=1))})

    nc = _build()

    trace = bool(int(os.environ.get("BASSKNN_TRACE", "0")))
    if trace:
        _install_ntff_hook()
    res = run_bass_kernel_spmd(nc, in_maps, list(range(N_CORES)), trace=trace)
    global LAST_EXEC_NS
    if trace:
        LAST_EXEC_NS = res.exec_time_ns

    full = np.empty((N, N), np.float32)
    for c in range(N_CORES):
        A = np.asarray(res.results[c]["outA"]).astype(np.float32)
        B = np.asarray(res.results[c]["outB"]).astype(np.float32)
        for t in range(9):
            j = (c + t) % NB
            blk = A[:, t * BS:(t + 1) * BS]
            full[c * BS:(c + 1) * BS, j * BS:(j + 1) * BS] = blk
            if t:
                full[j * BS:(j + 1) * BS, c * BS:(c + 1) * BS] = blk.T
        i2 = c + 8
        for e in range(8):
            j = (i2 + e) % NB
            blk = B[:, e * BS:(e + 1) * BS]
            full[i2 * BS:(i2 + 1) * BS, j * BS:(j + 1) * BS] = blk
            if e:
                full[j * BS:(j + 1) * BS, i2 * BS:(i2 + 1) * BS] = blk.T
    return full
